# revision 11
# baseline (speedup 1.0000x reference)
"""NsNet2 single-step (fc1 + 2x GRU cell + 3x FC) Trainium2 kernel.

Strategy:
  - Pure data parallel: batch B=32768 sharded as 4096 rows per NeuronCore (8 cores).
  - Feature-major ("transposed") layout on chip: activations live as [feat, batch]
    so every matmul's moving operand is already in [K, N] form -> zero on-chip
    transposes. Host transposes inputs/outputs (free; not on HW critical path).
  - ALL matmuls fp8 DoubleRow (GRU gates and the 3 FC layers) with fp32 PSUM
    accumulation. Measured on HW: a DR matmul (2 K-chunks) costs ~213 ns
    effective at nb=512 while a single-chunk fp8 matmul costs ~400 ns, so every
    K contraction is padded/split to an even chunk count (zero weights on pad
    chunks; operand pad chunks kept finite).
  - fc1 is folded into the GRU1 input-gate weights on the host (fc1 is linear and
    f1 is consumed only by GRU1's input matmuls):  (x@Wfc1.T+b) @ Wg.T =
    x @ (Wg@Wfc1).T + (Wg@b + bg).
  - GRU1 z,r gates sum their input-side and hidden-side matmuls in one PSUM over
    the K-concatenated [x|h1] operand (6 chunks -> 3 DR).
  - GRU2 z,r gates accumulate TWO matmul groups into one PSUM bank: g1-side
    (aligned g1 tile, 4 chunks -> 2 DR) then h2-side (aligned h2 fp8 tile,
    4 chunks -> 2 DR). No [g1|h2] concat assembly DMAs needed.
  - Elementwise fused across M-chunks: the n-gate preact lives in a [128,4,nb]
    tile (3 full chunks + 16-wide tail in chunk 3) -> ONE tanh, ONE d=h-n,
    ONE zd=z*d (z via stride-2 chunk view of the interleaved zro layout), ONE
    add writing the fp8 GRU output incl. zeroed pad rows (replaces memsets).
  - 3-stage software pipeline across batch tiles: per group emit
    A(t+2)=GRU1, B(t+1)=GRU2, C(t)=FC chain.
  - PSUM tags zr/nx/nh shared between GRU1 and GRU2 stages + fc tag: 4 tags x
    2 bufs = exactly 8 PSUM banks.
"""

import os
import sys

import numpy as np
import ml_dtypes

sys.path.insert(0, "/opt/trn_rl_repo")

import concourse.bacc as bacc
import concourse.bass as bass
import concourse.mybir as mybir
import concourse.tile as tile
from concourse.bass import ts
from concourse.bass_utils import run_bass_kernel_spmd

BF16 = ml_dtypes.bfloat16
FP8 = ml_dtypes.float8_e4m3

B, F, H, FF = 32768, 257, 400, 600
NCORES = 8
BPC = B // NCORES          # 4096 batch rows per core
Hp, FFp, Fp = 512, 640, 384  # padded feature dims
FFp6 = 768                 # fc3/fc4 K padded to 6 chunks (all-DR)
XH1 = 769                  # [x(257) | h1(400) | pad(112)] rows; 6 zr chunks + aligned h1 view at 257
ZRM = 880                  # 6x128 interleaved z/r chunks + 112-wide tail chunk
ZRC = 7
NB = 512                   # matmul free-dim tile (one PSUM bank of fp32)

AF = mybir.ActivationFunctionType
ALU = mybir.AluOpType

# packed bias column layout: name -> (offset, n_chunks)
# btl1/btl2: n-gate tail biases, partition-placed (64..79 = bnx tail,
# 96..111 = bnh tail; PSUM reads need 32-aligned partition bases) to line
# up with the zr chunk-6 tail matmul output.
BIAS_LAYOUT = {}
_off = 0
for _n, _c in (("bzr1", 7), ("bnx1", 4), ("bnh1", 4),
               ("bzr2", 7), ("bnx2", 4), ("bnh2", 4),
               ("bfc2", 5), ("bfc3", 5), ("bfc4", 3),
               ("btl1", 1), ("btl2", 1)):
    BIAS_LAYOUT[_n] = (_off, _c)
    _off += _c
BIAS_COLS = _off


def _pad2(a, rows, cols):
    out = np.zeros((rows, cols), dtype=np.float64)
    out[: a.shape[0], : a.shape[1]] = a
    return out


def _bias_tile(vec, padded):
    """Pack a [padded] bias vector as [128, padded//128] fp32 (partition-major)."""
    v = np.zeros(padded, dtype=np.float64)
    v[: vec.shape[0]] = vec
    return np.ascontiguousarray(v.reshape(padded // 128, 128).T).astype(np.float32)


def prepare_weights(inp):
    f64 = {k: np.asarray(v, dtype=np.float64) for k, v in inp.items()}
    w = {}

    # fc1 fold for GRU1 input side
    Wx = {}
    bx = {}
    for name in ("z", "r", "n"):
        Wx[name] = (f64[f"Wi{name}1"] @ f64["Wfc1"]).T          # [F, H]
        bx[name] = f64[f"bi{name}1"] + f64[f"Wi{name}1"] @ f64["bfc1"]

    # z,r gate M-layout: z and r 128-col chunks interleaved
    # [z0 r0 z1 r1 z2 r2 | ztail(16) rtail(16) nxtail(16) nhtail(16)] so both
    # gates' outputs are naturally 128-partition-aligned in PSUM/SBUF (no
    # on-chip realign), and the n-gate's 16-feature tail rides in the spare
    # M columns of the zr matmul's tail chunk.
    ZRPERM = np.concatenate([
        np.concatenate([np.arange(c * 128, (c + 1) * 128),
                        H + np.arange(c * 128, (c + 1) * 128)])
        for c in range(3)
    ] + [np.arange(384, 400), H + np.arange(384, 400)])

    # GRU1 z,r: K-concat [x(257) | h1(400)] -> rows 0..656 of XH1 space.
    tmp = np.zeros((768, 800), dtype=np.float64)
    for g, name in enumerate(("z", "r")):
        tmp[:F, g * H : g * H + H] = Wx[name]
        tmp[F : F + H, g * H : g * H + H] = f64[f"Wh{name}1"].T
    Wzr1p = np.zeros((768, ZRM), dtype=np.float64)
    Wzr1p[:, :800] = tmp[:, ZRPERM]
    # n-gate tails: cols 832..847 = nx tail (K = x rows), 864..879 = nh tail
    # (K = h1 rows at their xh positions)
    Wzr1p[:F, 832:848] = Wx["n"][:, 384:400]
    Wzr1p[F : F + H, 864:880] = f64["Whn1"].T[:, 384:400]
    w["Wzr1"] = Wzr1p
    # GRU1 n input side: K = xh chunks 0..3 (rows 257..511 are h1 -> zero W)
    w["Wn1x"] = _pad2(Wx["n"], Hp, Hp)
    # GRU1 n hidden side: aligned h1 (XH1 rows 257..768)
    w["Wn1h"] = _pad2(f64["Whn1"].T, Hp, Hp)

    # GRU2 z,r: split into g1-side and h2-side weight blocks; the two matmul
    # groups accumulate into the same PSUM bank.
    tmp2x = np.zeros((H, 800), dtype=np.float64)
    tmp2h = np.zeros((H, 800), dtype=np.float64)
    for g, name in enumerate(("z", "r")):
        tmp2x[:, g * H : g * H + H] = f64[f"Wi{name}2"].T
        tmp2h[:, g * H : g * H + H] = f64[f"Wh{name}2"].T
    Wzr2x = np.zeros((Hp, ZRM), dtype=np.float64)
    Wzr2x[:H, :800] = tmp2x[:, ZRPERM]
    Wzr2x[:H, 832:848] = f64["Win2"].T[:, 384:400]   # nx2 tail (K = g1)
    Wzr2h = np.zeros((Hp, ZRM), dtype=np.float64)
    Wzr2h[:H, :800] = tmp2h[:, ZRPERM]
    Wzr2h[:H, 864:880] = f64["Whn2"].T[:, 384:400]   # nh2 tail (K = h2)
    w["Wzr2x"] = Wzr2x
    w["Wzr2h"] = Wzr2h
    w["Wn2x"] = _pad2(f64["Win2"].T, Hp, Hp)
    w["Wn2h"] = _pad2(f64["Whn2"].T, Hp, Hp)

    w["Wfc2T"] = _pad2(f64["Wfc2"].T, Hp, FFp)     # [512, 640]
    w["Wfc3T"] = _pad2(f64["Wfc3"].T, FFp6, FFp)   # [768, 640]
    w["Wfc4T"] = _pad2(f64["Wfc4"].T, FFp6, Fp)    # [768, 384]

    weights = {
        k: np.ascontiguousarray(v).astype(FP8)
        for k, v in w.items()
    }

    parts = [
        ("bzr1", _bias_tile(np.concatenate([bx["z"] + f64["bhz1"],
                                            bx["r"] + f64["bhr1"]])[ZRPERM], 896)),
        ("bnx1", _bias_tile(bx["n"], Hp)),
        ("bnh1", _bias_tile(f64["bhn1"], Hp)),
        ("bzr2", _bias_tile(np.concatenate([f64["biz2"] + f64["bhz2"],
                                            f64["bir2"] + f64["bhr2"]])[ZRPERM], 896)),
        ("bnx2", _bias_tile(f64["bin2"], Hp)),
        ("bnh2", _bias_tile(f64["bhn2"], Hp)),
        ("bfc2", _bias_tile(f64["bfc2"], FFp)),
        ("bfc3", _bias_tile(f64["bfc3"], FFp)),
        ("bfc4", _bias_tile(f64["bfc4"], Fp)),
    ]

    def _tail_bias(bnx, bnh):
        v = np.zeros(128, dtype=np.float64)
        v[64:80] = bnx[384:400]
        v[96:112] = bnh[384:400]
        return v.reshape(128, 1).astype(np.float32)

    parts += [
        ("btl1", _tail_bias(bx["n"], f64["bhn1"])),
        ("btl2", _tail_bias(f64["bin2"], f64["bhn2"])),
    ]
    biases = {"biasT": np.concatenate([p[1] for p in parts], axis=1)}
    return weights, biases


def build_nc(nbt=BPC, nb=NB):
    """Build the per-core Bass program. nbt = per-core batch, nb = free-dim tile."""
    nc = bacc.Bacc("TRN2", target_bir_lowering=False, debug=False)
    bf = mybir.dt.bfloat16
    f32 = mybir.dt.float32

    f8 = mybir.dt.float8e4

    # xh8 rows: 0..256 = x.T, 257..656 = h1.T, 657..768 = zeros (fp8 matmul
    # operand). zr view = rows 0..767 (6 chunks); aligned-h1 view = 257..768.
    xh8 = nc.declare_dram_parameter("xh8", [XH1, nbt], f8, isOutput=False)
    h1T = nc.declare_dram_parameter("h1T", [Hp, nbt], bf, isOutput=False)
    h2T = nc.declare_dram_parameter("h2T", [Hp, nbt], bf, isOutput=False)
    # h28: fp8 h2, aligned; rhs for GRU2's zr h-side + nh matmuls.
    h28 = nc.declare_dram_parameter("h28", [Hp, nbt], f8, isOutput=False)
    wd = {}
    for name, k, m in (
        ("Wzr1", 768, ZRM), ("Wn1x", Hp, Hp), ("Wn1h", Hp, Hp),
        ("Wzr2x", Hp, ZRM), ("Wzr2h", Hp, ZRM),
        ("Wn2x", Hp, Hp), ("Wn2h", Hp, Hp),
        ("Wfc2T", Hp, FFp), ("Wfc3T", FFp6, FFp), ("Wfc4T", FFp6, Fp),
    ):
        wd[name] = nc.declare_dram_parameter(name, [k, m], f8, isOutput=False)
    biasT_d = nc.declare_dram_parameter("biasT", [128, BIAS_COLS], f32, isOutput=False)
    outT = nc.declare_dram_parameter("outT", [Fp, nbt], bf, isOutput=True)

    n_tiles = nbt // nb
    HC = Hp // 128  # 4 M-chunks per gate

    with tile.TileContext(nc) as tc:
        with (
            tc.tile_pool(name="wpool", bufs=1) as wpool,
            tc.tile_pool(name="bpool", bufs=1) as bpool,
            tc.tile_pool(name="io", bufs=3) as io,
            tc.tile_pool(name="cio", bufs=2) as cio,
            tc.tile_pool(name="inp", bufs=4) as inp,
            tc.tile_pool(name="act", bufs=3) as act,
            tc.tile_pool(name="psum", bufs=2, space="PSUM") as psum,
        ):
            # ACT-table warmup: first ScalarE transcendental carries the
            # ACT_TABLE_LOAD pseudo-inst; keep it off the critical chain.
            warm = bpool.tile([128, 1], f32, tag="warm")
            nc.vector.memset(warm, 0.0)
            nc.scalar.activation(warm, warm, AF.Sigmoid)
            warm2 = bpool.tile([128, 1], f32, tag="warm2")
            nc.vector.memset(warm2, 0.0)
            nc.scalar.activation(warm2, warm2, AF.Tanh)

            W, BT = {}, {}

            def load_w(name, eng, lo=0, hi=None):
                dram = wd[name]
                k, m = dram.shape
                if name in W:
                    t = W[name]
                else:
                    t = wpool.tile([128, k // 128, m], dram.dtype, tag=name,
                                   name=f"W{name}")
                    W[name] = t
                r = dram.rearrange("(c p) m -> p c m", p=128)
                hi = k // 128 if hi is None else hi
                eng.dma_start(out=t[:, lo:hi, :], in_=r[:, lo:hi, :])

            def load_bias():
                biasT = bpool.tile([128, BIAS_COLS], f32, tag="biasT")
                nc.scalar.dma_start(out=biasT, in_=biasT_d[:, :])
                for _n, (_o, _c) in BIAS_LAYOUT.items():
                    BT[_n] = biasT[:, _o : _o + _c]

            xh_zr = xh8[0:768, :].rearrange("(c p) n -> p c n", p=128)
            h1m_al = xh8[257 : 257 + Hp, :].rearrange("(c p) n -> p c n", p=128)
            h1_bl = h1T.rearrange("(c p) n -> p c n", p=128)
            h2_bl = h2T.rearrange("(c p) n -> p c n", p=128)
            h28_al = h28.rearrange("(c p) n -> p c n", p=128)
            outT_r = outT.rearrange("(c p) n -> p c n", p=128)

            def load_inputs(t, first=False):
                sl = ts(t, nb)
                xh = inp.tile([128, 6, nb], f8, tag="xh")      # zr1/nx1 K operand
                nc.sync.dma_start(out=xh, in_=xh_zr[:, :, sl])
                h1m = inp.tile([128, HC, nb], f8, tag="h1m")   # nh1 rhs (aligned h1)
                nc.sync.dma_start(out=h1m, in_=h1m_al[:, :, sl])
                if first:
                    # GRU1 weights land before the bulkier blend operands so
                    # tile-0 matmuls can start asap (DMA-capable queues are
                    # sync/scalar/gpsimd only).
                    load_w("Wzr1", nc.sync, 0, 3)
                    load_w("Wzr1", nc.gpsimd, 3, 6)
                    load_w("Wn1x", nc.scalar)
                    load_w("Wn1h", nc.scalar)
                h1s = inp.tile([128, HC, nb], bf, tag="h1s")   # blend h1
                nc.sync.dma_start(out=h1s, in_=h1_bl[:, :, sl])
                h2s = inp.tile([128, HC, nb], bf, tag="h2s")   # blend h2
                nc.sync.dma_start(out=h2s, in_=h2_bl[:, :, sl])
                h28s = inp.tile([128, HC, nb], f8, tag="h28s") # zr2-h/nh2 rhs
                nc.sync.dma_start(out=h28s, in_=h28_al[:, :, sl])
                return xh, h1m, h1s, h2s, h28s

            DR = mybir.MatmulPerfMode.DoubleRow

            def matseq_groups(ps, groups, col, mw):
                """Accumulate a list of (Wt, rhs_t, kc) matmul groups into one
                PSUM bank, all fp8 DoubleRow (every kc is even)."""
                total = sum(kc // 2 for _, _, kc in groups)
                i = 0
                for Wt, rhs_t, kc in groups:
                    assert kc % 2 == 0
                    for k in range(0, kc, 2):
                        nc.tensor.matmul(
                            ps, Wt[:, k : k + 2, col : col + mw],
                            rhs_t[:, k : k + 2, :],
                            start=(i == 0), stop=(i == total - 1), perf_mode=DR)
                        i += 1

            def gru(zr_groups, nx_groups, nh_groups, h_t4,
                    bzr, bnx, bnh, btl, out4):
                """One GRU step, all matmuls fp8/DoubleRow.
                zr/nx/nh_groups: list of (Wtile, rhs_tile[128,kc,nb], kc).
                h_t4: [128,4,nb] bf16 blend operand (pad rows zero).
                out4: [128,4,nb] fp8 output AP; chunks 0..2 + partitions 0..15
                of chunk 3 are written here, the rest is pre-zeroed (persistent
                tile).
                zr M layout (interleaved): z chunk m = zro[:, 2m], r chunk m =
                zro[:, 2m+1]; chunk 6 = [ztail(16) rtail(16) | raw n-gate tail
                preacts: nx(16)@64 nh(16)@96]."""
                zro = act.tile([128, ZRC, nb], bf, tag="zro", bufs=2)
                ps6 = None
                for c in range(ZRC):
                    mw = min(128, ZRM - c * 128)
                    ps = psum.tile([128, nb], f32, tag="ps_zr")
                    matseq_groups(ps[:mw, :], zr_groups, c * 128, mw)
                    sw = mw if c < 6 else 32
                    nc.scalar.activation(zro[:sw, c, :], ps[:sw, :], AF.Sigmoid,
                                         bias=bzr[:sw, c : c + 1])
                    if c == 6:
                        ps6 = ps

                # ---- 16-feature n-gate tail (features 384..399), folded into
                # the zr tail matmul; tiny lane-shift DMAs align operands. An
                # independent side path: only gates out4[0:16, 3, :].
                rtl = act.tile([112, nb], bf, tag="rtl")
                nc.gpsimd.dma_start(out=rtl[96:112, :], in_=zro[16:32, 6, :])
                rhn3 = act.tile([112, nb], f32, tag="rhn3")
                nc.vector.scalar_tensor_tensor(
                    rhn3[96:112, :], ps6[96:112, :], btl[96:112, 0:1],
                    rtl[96:112, :], op0=ALU.add, op1=ALU.mult)
                rhn3s = act.tile([80, nb], f32, tag="rhn3s")
                nc.gpsimd.dma_start(out=rhn3s[64:80, :], in_=rhn3[96:112, :])
                npre3 = act.tile([80, nb], f32, tag="npre3")
                nc.vector.scalar_tensor_tensor(
                    npre3[64:80, :], ps6[64:80, :], btl[64:80, 0:1],
                    rhn3s[64:80, :], op0=ALU.add, op1=ALU.add)
                n3 = act.tile([80, nb], bf, tag="n3")
                nc.scalar.activation(n3[64:80, :], npre3[64:80, :], AF.Tanh)
                n3s = act.tile([16, nb], bf, tag="n3s")
                nc.gpsimd.dma_start(out=n3s, in_=n3[64:80, :])
                d3 = act.tile([16, nb], bf, tag="d3")
                nc.vector.tensor_sub(d3, h_t4[0:16, 3, :], n3s)
                zd3 = act.tile([16, nb], bf, tag="zd3")
                nc.vector.tensor_mul(zd3, zro[0:16, 6, :], d3)
                nc.vector.tensor_add(out4[0:16, 3, :], n3s, zd3)

                # ---- full 128-feature n-gate chunks ----
                npre = act.tile([128, 3, nb], f32, tag="npre", bufs=2)
                for m in range(3):
                    col = m * 128
                    psx = psum.tile([128, nb], f32, tag="ps_nx")
                    matseq_groups(psx, nx_groups, col, 128)
                    psh = psum.tile([128, nb], f32, tag="ps_nh")
                    matseq_groups(psh, nh_groups, col, 128)
                    # rhn = (psh + bnh) * r ; npre = (psx + bnx) + rhn
                    rhn = act.tile([128, nb], f32, tag="rhn", bufs=2)
                    nc.vector.scalar_tensor_tensor(
                        rhn, psh, bnh[:, m : m + 1], zro[:, 2 * m + 1, :],
                        op0=ALU.add, op1=ALU.mult)
                    nc.vector.scalar_tensor_tensor(
                        npre[:, m, :], psx, bnx[:, m : m + 1], rhn,
                        op0=ALU.add, op1=ALU.add)

                # ---- fused over the 3 full chunks ----
                n_t = act.tile([128, 3, nb], bf, tag="n_t", bufs=2)
                nc.scalar.activation(n_t, npre, AF.Tanh)
                # h' = n + z*(h - n)
                d = act.tile([128, 3, nb], bf, tag="d", bufs=2)
                nc.vector.tensor_sub(d, h_t4[:, 0:3, :], n_t)
                zd = act.tile([128, 3, nb], bf, tag="zd", bufs=2)
                nc.vector.tensor_mul(zd, zro[:, 0:6:2, :], d)
                # final blend add on GpSimd: SBUF-only op, off the busy DVE;
                # its latency is hidden by a full pipeline group of slack.
                nc.gpsimd.tensor_add(out4[:, 0:3, :], n_t, zd)

            def fc(outs, in_t, kc, Wt, bias, mc, kind, all_scalar=False):
                for m in range(mc):
                    ps = psum.tile([128, nb], f32, tag="ps_fc")
                    matseq_groups(ps, [(Wt, in_t, kc)], m * 128, 128)
                    if kind == "relu":
                        # relu split between ScalarE and DVE to balance load
                        if all_scalar or m % 2 == 0:
                            nc.scalar.activation(outs[:, m, :], ps, AF.Relu,
                                                 bias=bias[:, m : m + 1])
                        else:
                            nc.vector.tensor_scalar(
                                outs[:, m, :], ps, bias[:, m : m + 1], 0.0,
                                op0=ALU.add, op1=ALU.max)
                    else:
                        nc.scalar.activation(outs[:, m, :], ps, AF.Sigmoid,
                                             bias=bias[:, m : m + 1])
                return outs

            # ---- persistent double-buffered tiles with pre-zeroed pad lanes
            # (written once here; per-tile writes never touch them). Engine
            # APs with a nonzero partition base must stay inside one
            # 32-partition quadrant, hence the split memsets.
            g1_t, g2_t, f2_t, f3_t = [], [], [], []
            for i in range(2):
                g1_p = io.tile([128, HC, nb], f8, tag=f"g1_{i}", bufs=1)
                for q in (0, 32, 64, 96):
                    nc.vector.memset(g1_p[q : q + 32, 3, :], 0.0)
                g1_t.append(g1_p)
                g2_p = io.tile([128, HC, nb], f8, tag=f"g2_{i}", bufs=1)
                for q in (0, 32, 64, 96):
                    nc.vector.memset(g2_p[q : q + 32, 3, :], 0.0)
                g2_t.append(g2_p)
                f2_p = cio.tile([128, 6, nb], f8, tag=f"f2_{i}", bufs=1)
                nc.vector.memset(f2_p[:, 5, :], 0.0)
                f2_t.append(f2_p)
                f3_p = cio.tile([128, 6, nb], f8, tag=f"f3_{i}", bufs=1)
                nc.vector.memset(f3_p[:, 5, :], 0.0)
                f3_t.append(f3_p)

            # ---- pipeline stages ----
            st = {}  # t -> dict of live tiles

            def stage_A(t):
                """GRU1 for tile t: consumes inputs, produces the aligned fp8
                g1 tile (rhs for GRU2's zr-g1-side and nx matmuls)."""
                xh, h1m, h1s, h2s, h28s = st[t]["in"]
                g1t = g1_t[t % 2]
                gru([(W["Wzr1"], xh, 6)], [(W["Wn1x"], xh, HC)],
                    [(W["Wn1h"], h1m, HC)], h1s,
                    BT["bzr1"], BT["bnx1"], BT["bnh1"], BT["btl1"], g1t)
                st[t]["g1t"] = g1t

            def stage_B(t):
                """GRU2 for tile t: consumes g1t + h2 operands, produces g2
                (fp8, fc2's K operand)."""
                g1t = st[t]["g1t"]
                _, _, _, h2s, h28s = st[t]["in"]
                g2 = g2_t[t % 2]
                gru([(W["Wzr2x"], g1t, HC), (W["Wzr2h"], h28s, HC)],
                    [(W["Wn2x"], g1t, HC)], [(W["Wn2h"], h28s, HC)], h2s,
                    BT["bzr2"], BT["bnx2"], BT["bnh2"], BT["btl2"], g2)
                st[t]["g2"] = g2

            def stage_C(t):
                """FC chain for tile t: g2 -> relu fc2 -> relu fc3 -> sigmoid
                fc4 -> DMA out."""
                sl = ts(t, nb)
                g2 = st[t]["g2"]
                f2 = fc(f2_t[t % 2], g2, HC, W["Wfc2T"], BT["bfc2"],
                        FFp // 128, "relu", all_scalar=True)
                f3 = fc(f3_t[t % 2], f2, 6, W["Wfc3T"], BT["bfc3"],
                        FFp // 128, "relu")
                o = cio.tile([128, Fp // 128, nb], bf, tag="o")
                fc(o, f3, 6, W["Wfc4T"], BT["bfc4"], Fp // 128, "sig")
                nc.sync.dma_start(out=outT_r[:, :, sl], in_=o)
                del st[t]

            # ---- emission: software-pipelined interleave ----
            # prologue
            st[0] = {"in": load_inputs(0, first=True)}
            load_bias()
            # GRU2 weights stream on the gpsimd queue, which is idle early
            load_w("Wzr2x", nc.gpsimd)
            load_w("Wzr2h", nc.gpsimd)
            load_w("Wn2x", nc.gpsimd)
            load_w("Wn2h", nc.gpsimd)
            stage_A(0)
            st[1] = {"in": load_inputs(1)}
            # fc weights interleave on the scalar queue between early stages:
            # behind tile-0/1 activations (so they don't delay them) but well
            # before C(0) consumes them.
            load_w("Wfc2T", nc.scalar)
            stage_A(1)
            st[2] = {"in": load_inputs(2)}
            load_w("Wfc3T", nc.scalar)
            stage_B(0)
            load_w("Wfc4T", nc.scalar)
            # steady state: per group emit A(t+2), B(t+1), C(t)
            for t in range(n_tiles - 2):
                if t + 3 < n_tiles:
                    st[t + 3] = {"in": load_inputs(t + 3)}
                stage_A(t + 2)
                stage_B(t + 1)
                stage_C(t)
            # epilogue
            stage_B(n_tiles - 1)
            stage_C(n_tiles - 2)
            stage_C(n_tiles - 1)

    nc.compile()
    return nc


def _shard_inputs(inp, weights, biases):
    x = np.asarray(inp["x"], dtype=np.float32)
    h1 = np.asarray(inp["h1"], dtype=np.float32)
    h2 = np.asarray(inp["h2"], dtype=np.float32)

    xh8 = np.zeros((NCORES, XH1, BPC), dtype=FP8)    # matmul operand [x|h1]
    h1T = np.zeros((NCORES, Hp, BPC), dtype=BF16)    # blend h1
    h2T = np.zeros((NCORES, Hp, BPC), dtype=BF16)    # blend h2
    h28 = np.zeros((NCORES, Hp, BPC), dtype=FP8)     # matmul h2
    for i in range(NCORES):
        sl = slice(i * BPC, (i + 1) * BPC)
        xh8[i, :F] = x[sl].T.astype(FP8)
        xh8[i, F : F + H] = h1[sl].T.astype(FP8)
        h1T[i, :H] = h1[sl].T.astype(BF16)
        h2T[i, :H] = h2[sl].T.astype(BF16)
        h28[i, :H] = h2[sl].T.astype(FP8)

    in_maps = []
    for i in range(NCORES):
        m = {"xh8": xh8[i], "h1T": h1T[i], "h2T": h2T[i], "h28": h28[i]}
        m.update(weights)
        m.update(biases)
        in_maps.append(m)
    return in_maps


def _run(inp, trace=False):
    weights, biases = prepare_weights(inp)
    nc = build_nc()
    in_maps = _shard_inputs(inp, weights, biases)
    res = run_bass_kernel_spmd(nc, in_maps, list(range(NCORES)), trace=trace)
    out = np.empty((B, F), dtype=np.float32)
    for i in range(NCORES):
        out[i * BPC : (i + 1) * BPC] = (
            np.asarray(res.results[i]["outT"][:F]).astype(np.float32).T
        )
    return out, res


def kernel(**inputs) -> np.ndarray:
    out, _ = _run(inputs, trace=False)
    return out
